# revision 8
# baseline (speedup 1.0000x reference)
"""Trainium2 8-core kernel for nn_AlignedGloveLayer (retrieval 1-NN mismatch loss).

Problem: a = mapped[indexes] ([4096, 256]); d2[k, j] = |a_k - target_j|^2 over
30000 targets; loss = mean over k of (argmin_j d2[k, j] != indexes[k]).

Strategy (witness counting): query k is mismatched iff SOME target j has
d2[k, j] < d2[k, indexes[k]]. The device searches a fixed sampled subset of
S targets for witnesses with margin DELTA (covering all device arithmetic
error): any witness found proves mismatch; queries with no witness are
resolved exactly on the host (a handful for random data, since a query's
own-index distance typically ranks ~uniformly among 30000 distances).

The sampled subset is the S targets whose squared norms b2 are CLOSEST TO THE
MEDIAN b2. Within that band b2_j = B2C +- HW with HW ~2, so b2 folds into the
per-query threshold (widened by HW) and the device never touches b2 at all:
  witness claim:  -2 a_k . t_j < v_k - B2C - (DELTA + HW)
  soundness:      d2_jk = b2_j - 2 a.t < B2C + HW + v_k - B2C - DELTA - HW
                        = v_k - DELTA  (true closer target)

Device layout (queries on PSUM partitions, targets on the free dim):
  2x4 grid: cores 0-3 take 1024 queries each over the first S/2 band targets;
  cores 4-7 the same query slices over the second S/2. Per core, 8 query
  blocks of 128; per block one PSUM tile [128, S_c]:
    psum[q, t] = sum_d T[t, d] * (-2 a[q, d])   (fp8 DoubleRow, 256-deep)
  then ONE fused instruction per tile yields the per-query witness measure:
    ACT: out = Relu(thr_q - psum), accum_out[q] = sum(out)   (>0 iff witness)
    DVE: out = (psum is_lt thr_q), accum_out[q] = count
  Only the [128, 8] accum table is DMA'd out (4KB/core).
"""
import os
import sys

for _p in ("/opt/trn_rl_repo", "/root/.axon_site/_ro/trn_rl_repo"):
    if os.path.isdir(_p) and _p not in sys.path:
        sys.path.append(_p)

from contextlib import ExitStack

import ml_dtypes
import numpy as np

NX, NY, D, K = 30000, 30000, 256, 4096
NCORES = 8
P = 128
DC = D // P          # 2 contraction k-tiles (fp8 DoubleRow: 256-deep)
NQ = 1024            # queries per core (cores c and c+4 share a query slice)
QB = NQ // P         # 8 query blocks
S_TOTAL = 1024       # sampled targets (device witness search set)
S_C = S_TOTAL // 2   # sampled targets per core (two halves)
DELTA = 18.5         # witness margin >= device arithmetic error bound
ACT_SET = (1, 3, 5, 7)  # query blocks routed through ScalarE

_CACHE: dict = {}


def _build_nc():
    import concourse.tile as tile
    from concourse import bacc, mybir
    nc = bacc.Bacc("TRN2", target_bir_lowering=False)
    at_d = nc.dram_tensor("at", [P, DC, NQ], mybir.dt.float8e4, kind="ExternalInput")
    tt_d = nc.dram_tensor("tt", [P, DC, S_C], mybir.dt.float8e4, kind="ExternalInput")
    vb_d = nc.dram_tensor("vb", [P, QB], mybir.dt.float32, kind="ExternalInput")
    accw_d = nc.dram_tensor("accw", [P, QB], mybir.dt.float32, kind="ExternalOutput")

    with tile.TileContext(nc) as tc:
        with ExitStack() as ctx:
            sb = ctx.enter_context(tc.tile_pool(name="sb", bufs=1))
            dump = ctx.enter_context(tc.tile_pool(name="dump", bufs=3))
            nbanks = max(1, S_C // 512)
            psum = ctx.enter_context(
                tc.tile_pool(name="psum", bufs=8 // nbanks, space="PSUM")
            )

            # Inputs spread across the three DMA-capable queues (SP, gpsimd,
            # Activation), ordered so the first matmul's deps land first.
            accw = sb.tile([P, QB], mybir.dt.float32)
            nc.gpsimd.memset(accw[:], 0.0)
            warm = sb.tile([P, 1], mybir.dt.float32)
            nc.gpsimd.memset(warm[:], 0.0)
            tt = sb.tile([P, DC, S_C], mybir.dt.float8e4)
            nc.sync.dma_start(tt[:], tt_d[:])
            at = sb.tile([P, DC, NQ], mybir.dt.float8e4)
            nc.gpsimd.dma_start(at[:, :, :NQ // 2], at_d[:, :, :NQ // 2])
            vb = sb.tile([P, QB], mybir.dt.float32)
            nc.sync.dma_start(vb[:], vb_d[:])
            zz = sb.tile([P, S_C], mybir.dt.bfloat16)
            nc.vector.memset(zz[:], 0.0)

            # Pull the ACT table load off the critical path while DMAs fly;
            # the second half of `at` (query blocks 4-7) follows it on the
            # Activation queue.
            nc.scalar.activation(
                warm[:], warm[:], mybir.ActivationFunctionType.Relu,
                bias=0.0, scale=1.0,
            )
            nc.scalar.dma_start(at[:, :, NQ // 2:], at_d[:, :, NQ // 2:])

            for qb in range(QB):
                ps = psum.tile([P, S_C], mybir.dt.float32)
                for h in range(max(1, S_C // 512)):
                    t0 = h * 512
                    tw = min(512, S_C)
                    nc.tensor.matmul(
                        ps[:, t0:t0 + tw],
                        at[:, :, qb * P:(qb + 1) * P],
                        tt[:, :, t0:t0 + tw],
                        start=True, stop=True,
                        perf_mode=mybir.MatmulPerfMode.DoubleRow,
                    )
                vo = dump.tile([P, S_C], mybir.dt.float16, tag="vo")
                if qb in ACT_SET:
                    # accum[q] = sum_t relu(thr_q - psum) : > 0 iff witness
                    nc.scalar.activation(
                        vo[:], ps[:], mybir.ActivationFunctionType.Relu,
                        bias=vb[:, qb:qb + 1], scale=-1.0,
                        accum_out=accw[:, qb:qb + 1],
                    )
                else:
                    # accum[q] = #targets with (psum - thr_q) < 0
                    nc.vector.scalar_tensor_tensor(
                        vo[:], ps[:], vb[:, qb:qb + 1], zz[:],
                        op0=mybir.AluOpType.subtract,
                        op1=mybir.AluOpType.is_lt,
                        accum_out=accw[:, qb:qb + 1],
                    )
                # stream each block's accum out as soon as it lands, so only
                # the last small DMA sits on the tail
                nc.sync.dma_start(accw_d[:, qb:qb + 1], accw[:, qb:qb + 1])

    nc.compile()
    return nc


def _get_nc():
    if "nc" not in _CACHE:
        _CACHE["nc"] = _build_nc()
    return _CACHE["nc"]


def _marshal(mapped, target, idx):
    """Host-side sharding/quantization. Returns (in_maps, a, b2_64)."""
    a = mapped[idx]                                   # [K, D] fp32
    at_all = np.ascontiguousarray((-2.0 * a).T)       # [D, K]

    b2_64 = (target.astype(np.float64) ** 2).sum(1)   # exact fp64 row norms
    med = np.median(b2_64)
    sidx = np.sort(np.argsort(np.abs(b2_64 - med))[:S_TOTAL])
    _CACHE["sidx"] = sidx
    b2band = b2_64[sidx]
    b2c = float(b2band.mean())
    hw = float(np.abs(b2band - b2c).max())            # band halfwidth
    _CACHE["band"] = (b2c, hw)
    tsub = target[sidx]                               # [S, D]

    # tt[p, dc, t] = tsub[t, dc*128 + p] in fp8
    tt_all = np.ascontiguousarray(
        tsub.reshape(S_TOTAL, DC, P).transpose(2, 1, 0)
    ).astype(ml_dtypes.float8_e4m3)                   # [P, DC, S]
    tt_half = [np.ascontiguousarray(tt_all[:, :, :S_C]),
               np.ascontiguousarray(tt_all[:, :, S_C:])]

    # v_k = d2 at own index (exact); thr = v - b2c - (DELTA + hw)
    v = b2_64[idx] - 2.0 * np.einsum(
        "kd,kd->k", a.astype(np.float64), target[idx].astype(np.float64)
    )
    _CACHE["v"] = v
    thr_all = (v - b2c - (DELTA + hw)).astype(np.float32)

    at_cores, vb_cores = [], []
    for cq in range(K // NQ):                          # 4 query slices
        sl = slice(cq * NQ, (cq + 1) * NQ)
        at_cores.append(np.ascontiguousarray(
            at_all[:, sl].reshape(DC, P, NQ).transpose(1, 0, 2)
        ).astype(ml_dtypes.float8_e4m3))               # [P, DC, NQ]
        vb_cores.append(np.ascontiguousarray(thr_all[sl].reshape(QB, P).T))

    in_maps = []
    for c in range(NCORES):
        half, cq = c // 4, c % 4
        in_maps.append({
            "at": at_cores[cq], "tt": tt_half[half], "vb": vb_cores[cq],
        })
    return in_maps, a, b2_64


def kernel(mapped: np.ndarray, target: np.ndarray, indexes: np.ndarray) -> np.ndarray:
    from concourse.bass_utils import run_bass_kernel_spmd

    mapped = np.asarray(mapped, dtype=np.float32)
    target = np.asarray(target, dtype=np.float32)
    idx = np.asarray(indexes).astype(np.int64)

    in_maps, a, b2_64 = _marshal(mapped, target, idx)

    # ---- run on the 8 NeuronCores (host numpy fallback if the device path
    # fails repeatedly — correctness insurance) ----
    witness = None
    last_exc = None
    for attempt in range(3):
        try:
            nc = _get_nc()
            kwargs = {}
            if os.environ.get("KERNEL_TRACE_DIR"):
                kwargs["tmpdir"] = os.environ["KERNEL_TRACE_DIR"]
            res = run_bass_kernel_spmd(
                nc, in_maps, core_ids=list(range(NCORES)), **kwargs
            )
            _CACHE["last_res"] = res  # exec_time_ns/profile when BASS_TRACE=1
            # accw[p, qb] on core c: measure for query (c%4)*1024 + qb*128 + p
            # over the sampled-target half c//4
            w = np.zeros(K, dtype=np.float64)
            for c in range(NCORES):
                acc = res.results[c]["accw"].astype(np.float64)  # [P, QB]
                cq = c % 4
                w[cq * NQ:(cq + 1) * NQ] += acc.T.reshape(NQ)
            witness = w > 0.0
            break
        except Exception as e:  # noqa: BLE001 - retry/fallback on any device error
            last_exc = e
            _CACHE.pop("nc", None)
    if witness is None:
        sys.stderr.write(f"kernel: device path failed ({last_exc}); host fallback\n")
        witness = np.zeros(K, dtype=bool)

    # ---- host decision: witnessed queries are proven mismatched; the rest
    # get an exact fp64 check ----
    mismatch = witness.copy()
    flagged = np.nonzero(~witness)[0]
    _CACHE["flagged_n"] = len(flagged)
    t64 = None
    for i in range(0, len(flagged), 64):
        blk = flagged[i:i + 64]
        if t64 is None:
            t64 = target.astype(np.float64)
        d2 = b2_64[None, :] - 2.0 * (a[blk].astype(np.float64) @ t64.T)
        mismatch[blk] = np.argmin(d2, axis=1) != idx[blk]

    return np.asarray(mismatch.mean(), dtype=np.float32)


if __name__ == "__main__":
    rng = np.random.default_rng(1)
    mapped = rng.standard_normal((NX, D)).astype(np.float32)
    target = rng.standard_normal((NY, D)).astype(np.float32)
    indexes = rng.integers(0, NY, size=K).astype(np.int32)
    out = kernel(mapped=mapped, target=target, indexes=indexes)
    print("kernel output:", out, out.shape, out.dtype)


# revision 10
# speedup vs baseline: 1.1648x; 1.1648x over previous
"""Trainium2 8-core kernel for nn_AlignedGloveLayer (retrieval 1-NN mismatch loss).

Problem: a = mapped[indexes] ([4096, 256]); d2[k, j] = |a_k - target_j|^2 over
30000 targets; loss = mean over k of (argmin_j d2[k, j] != indexes[k]).

Strategy (witness counting): query k is mismatched iff SOME target j has
d2[k, j] < d2[k, indexes[k]]. The device searches a fixed sampled subset of
S targets for witnesses with margin DELTA (covering all device arithmetic
error): any witness found proves mismatch; queries with no witness are
resolved exactly on the host (a handful for random data, since a query's
own-index distance typically ranks ~uniformly among 30000 distances).

The sampled subset is the S targets whose squared norms b2 are CLOSEST TO THE
MEDIAN b2. Within that band b2_j = B2C +- HW with HW ~2, so b2 folds into the
per-query threshold (widened by HW) and the device never touches b2 at all:
  witness claim:  -2 a_k . t_j < v_k - B2C - (DELTA + HW)
  soundness:      d2_jk = b2_j - 2 a.t < B2C + HW + v_k - B2C - DELTA - HW
                        = v_k - DELTA  (true closer target)

Device layout (queries on PSUM partitions, targets on the free dim):
  2x4 grid: cores 0-3 take 1024 queries each over the first S/2 band targets;
  cores 4-7 the same query slices over the second S/2. Per core, 8 query
  blocks of 128; per block one PSUM tile [128, S_c]:
    psum[q, t] = sum_d T[t, d] * (-2 a[q, d])   (fp8 DoubleRow, 256-deep)
  then ONE fused instruction per tile yields the per-query witness measure:
    ACT: out = Relu(thr_q - psum), accum_out[q] = sum(out)   (>0 iff witness)
    DVE: out = (psum is_lt thr_q), accum_out[q] = count
  Only the [128, 8] accum table is DMA'd out (4KB/core).
"""
import os
import sys

for _p in ("/opt/trn_rl_repo", "/root/.axon_site/_ro/trn_rl_repo"):
    if os.path.isdir(_p) and _p not in sys.path:
        sys.path.append(_p)

from contextlib import ExitStack

import ml_dtypes
import numpy as np

NX, NY, D, K = 30000, 30000, 256, 4096
NCORES = 8
P = 128
DC = D // P          # 2 contraction k-tiles (fp8 DoubleRow: 256-deep)
NQ = 1024            # queries per core (cores c and c+4 share a query slice)
QB = NQ // P         # 8 query blocks
S_TOTAL = 1024       # sampled targets (device witness search set)
S_C = S_TOTAL // 2   # sampled targets per core (two halves)
DELTA = 18.5         # witness margin >= device arithmetic error bound
ACT_SET = (1, 3, 5, 7)  # query blocks routed through ScalarE

_CACHE: dict = {}


def _build_nc():
    import concourse.tile as tile
    from concourse import bacc, mybir
    nc = bacc.Bacc("TRN2", target_bir_lowering=False)
    at_d = nc.dram_tensor("at", [P, DC, NQ], mybir.dt.float8e4, kind="ExternalInput")
    tt_d = nc.dram_tensor("tt", [P, DC, S_C], mybir.dt.float8e4, kind="ExternalInput")
    vb_d = nc.dram_tensor("vb", [P, QB], mybir.dt.float32, kind="ExternalInput")
    accw_d = nc.dram_tensor("accw", [P, QB], mybir.dt.float32, kind="ExternalOutput")

    with tile.TileContext(nc) as tc:
        with ExitStack() as ctx:
            sb = ctx.enter_context(tc.tile_pool(name="sb", bufs=1))
            dump = ctx.enter_context(tc.tile_pool(name="dump", bufs=3))
            nbanks = max(1, S_C // 512)
            psum = ctx.enter_context(
                tc.tile_pool(name="psum", bufs=8 // nbanks, space="PSUM")
            )

            # Inputs spread across the three DMA-capable queues (SP, gpsimd,
            # Activation), ordered so the first matmul's deps land first.
            tt = sb.tile([P, DC, S_C], mybir.dt.float8e4)
            nc.sync.dma_start(tt[:], tt_d[:])
            at = sb.tile([P, DC, NQ], mybir.dt.float8e4)
            nc.gpsimd.dma_start(at[:, :, :NQ // 2], at_d[:, :, :NQ // 2])
            vb = sb.tile([P, QB], mybir.dt.float32)
            nc.sync.dma_start(vb[:], vb_d[:])
            accw = sb.tile([P, QB], mybir.dt.float32)
            nc.gpsimd.memset(accw[:], 0.0)
            warm = sb.tile([P, 1], mybir.dt.float32)
            nc.gpsimd.memset(warm[:], 0.0)
            zz = sb.tile([P, S_C], mybir.dt.bfloat16)
            nc.vector.memset(zz[:], 0.0)

            # Pull the ACT table load off the critical path while DMAs fly;
            # the second half of `at` (query blocks 4-7) follows it on the
            # Activation queue.
            nc.scalar.activation(
                warm[:], warm[:], mybir.ActivationFunctionType.Relu,
                bias=0.0, scale=1.0,
            )
            nc.scalar.dma_start(at[:, :, NQ // 2:], at_d[:, :, NQ // 2:])

            for qb in range(QB):
                ps = psum.tile([P, S_C], mybir.dt.float32)
                for h in range(max(1, S_C // 512)):
                    t0 = h * 512
                    tw = min(512, S_C)
                    nc.tensor.matmul(
                        ps[:, t0:t0 + tw],
                        at[:, :, qb * P:(qb + 1) * P],
                        tt[:, :, t0:t0 + tw],
                        start=True, stop=True,
                        perf_mode=mybir.MatmulPerfMode.DoubleRow,
                    )
                vo = dump.tile([P, S_C], mybir.dt.float16, tag="vo")
                if qb in ACT_SET:
                    # accum[q] = sum_t relu(thr_q - psum) : > 0 iff witness
                    nc.scalar.activation(
                        vo[:], ps[:], mybir.ActivationFunctionType.Relu,
                        bias=vb[:, qb:qb + 1], scale=-1.0,
                        accum_out=accw[:, qb:qb + 1],
                    )
                else:
                    # accum[q] = #targets with (psum - thr_q) < 0
                    nc.vector.scalar_tensor_tensor(
                        vo[:], ps[:], vb[:, qb:qb + 1], zz[:],
                        op0=mybir.AluOpType.subtract,
                        op1=mybir.AluOpType.is_lt,
                        accum_out=accw[:, qb:qb + 1],
                    )
            nc.sync.dma_start(accw_d[:], accw[:])

    nc.compile()
    return nc


def _get_nc():
    if "nc" not in _CACHE:
        _CACHE["nc"] = _build_nc()
    return _CACHE["nc"]


def _marshal(mapped, target, idx):
    """Host-side sharding/quantization. Returns (in_maps, a, b2_64)."""
    a = mapped[idx]                                   # [K, D] fp32
    at_all = np.ascontiguousarray((-2.0 * a).T)       # [D, K]

    b2_64 = (target.astype(np.float64) ** 2).sum(1)   # exact fp64 row norms
    med = np.median(b2_64)
    sidx = np.sort(np.argsort(np.abs(b2_64 - med))[:S_TOTAL])
    _CACHE["sidx"] = sidx
    b2band = b2_64[sidx]
    b2c = float(b2band.mean())
    hw = float(np.abs(b2band - b2c).max())            # band halfwidth
    _CACHE["band"] = (b2c, hw)
    tsub = target[sidx]                               # [S, D]

    # tt[p, dc, t] = tsub[t, dc*128 + p] in fp8
    tt_all = np.ascontiguousarray(
        tsub.reshape(S_TOTAL, DC, P).transpose(2, 1, 0)
    ).astype(ml_dtypes.float8_e4m3)                   # [P, DC, S]
    tt_half = [np.ascontiguousarray(tt_all[:, :, :S_C]),
               np.ascontiguousarray(tt_all[:, :, S_C:])]

    # v_k = d2 at own index (exact); thr = v - b2c - (DELTA + hw)
    v = b2_64[idx] - 2.0 * np.einsum(
        "kd,kd->k", a.astype(np.float64), target[idx].astype(np.float64)
    )
    _CACHE["v"] = v
    thr_all = (v - b2c - (DELTA + hw)).astype(np.float32)

    at_cores, vb_cores = [], []
    for cq in range(K // NQ):                          # 4 query slices
        sl = slice(cq * NQ, (cq + 1) * NQ)
        at_cores.append(np.ascontiguousarray(
            at_all[:, sl].reshape(DC, P, NQ).transpose(1, 0, 2)
        ).astype(ml_dtypes.float8_e4m3))               # [P, DC, NQ]
        vb_cores.append(np.ascontiguousarray(thr_all[sl].reshape(QB, P).T))

    in_maps = []
    for c in range(NCORES):
        half, cq = c // 4, c % 4
        in_maps.append({
            "at": at_cores[cq], "tt": tt_half[half], "vb": vb_cores[cq],
        })
    return in_maps, a, b2_64


def kernel(mapped: np.ndarray, target: np.ndarray, indexes: np.ndarray) -> np.ndarray:
    from concourse.bass_utils import run_bass_kernel_spmd

    mapped = np.asarray(mapped, dtype=np.float32)
    target = np.asarray(target, dtype=np.float32)
    idx = np.asarray(indexes).astype(np.int64)

    in_maps, a, b2_64 = _marshal(mapped, target, idx)

    # ---- run on the 8 NeuronCores (host numpy fallback if the device path
    # fails repeatedly — correctness insurance) ----
    witness = None
    last_exc = None
    for attempt in range(3):
        try:
            nc = _get_nc()
            kwargs = {}
            if os.environ.get("KERNEL_TRACE_DIR"):
                kwargs["tmpdir"] = os.environ["KERNEL_TRACE_DIR"]
            res = run_bass_kernel_spmd(
                nc, in_maps, core_ids=list(range(NCORES)), **kwargs
            )
            _CACHE["last_res"] = res  # exec_time_ns/profile when BASS_TRACE=1
            # accw[p, qb] on core c: measure for query (c%4)*1024 + qb*128 + p
            # over the sampled-target half c//4
            w = np.zeros(K, dtype=np.float64)
            for c in range(NCORES):
                acc = res.results[c]["accw"].astype(np.float64)  # [P, QB]
                cq = c % 4
                w[cq * NQ:(cq + 1) * NQ] += acc.T.reshape(NQ)
            witness = w > 0.0
            break
        except Exception as e:  # noqa: BLE001 - retry/fallback on any device error
            last_exc = e
            _CACHE.pop("nc", None)
    if witness is None:
        sys.stderr.write(f"kernel: device path failed ({last_exc}); host fallback\n")
        witness = np.zeros(K, dtype=bool)

    # ---- host decision: witnessed queries are proven mismatched; the rest
    # get an exact fp64 check ----
    mismatch = witness.copy()
    flagged = np.nonzero(~witness)[0]
    _CACHE["flagged_n"] = len(flagged)
    t64 = None
    for i in range(0, len(flagged), 64):
        blk = flagged[i:i + 64]
        if t64 is None:
            t64 = target.astype(np.float64)
        d2 = b2_64[None, :] - 2.0 * (a[blk].astype(np.float64) @ t64.T)
        mismatch[blk] = np.argmin(d2, axis=1) != idx[blk]

    return np.asarray(mismatch.mean(), dtype=np.float32)


if __name__ == "__main__":
    rng = np.random.default_rng(1)
    mapped = rng.standard_normal((NX, D)).astype(np.float32)
    target = rng.standard_normal((NY, D)).astype(np.float32)
    indexes = rng.integers(0, NY, size=K).astype(np.int32)
    out = kernel(mapped=mapped, target=target, indexes=indexes)
    print("kernel output:", out, out.shape, out.dtype)


# revision 15
# speedup vs baseline: 1.1949x; 1.0258x over previous
"""Trainium2 8-core kernel for nn_AlignedGloveLayer (retrieval 1-NN mismatch loss).

Problem: a = mapped[indexes] ([4096, 256]); d2[k, j] = |a_k - target_j|^2 over
30000 targets; loss = mean over k of (argmin_j d2[k, j] != indexes[k]).

Strategy (witness counting): query k is mismatched iff SOME target j has
d2[k, j] < d2[k, indexes[k]]. The device searches a fixed sampled subset of
S targets for witnesses with margin DELTA (covering all device arithmetic
error): any witness found proves mismatch; queries with no witness are
resolved exactly on the host (a handful for random data, since a query's
own-index distance typically ranks ~uniformly among 30000 distances).

The sampled subset is the S targets whose squared norms b2 are CLOSEST TO THE
MEDIAN b2. Within that band b2_j = B2C +- HW with HW ~2, so b2 folds into the
per-query threshold (widened by HW) and the device never touches b2 at all:
  witness claim:  -2 a_k . t_j < v_k - B2C - (DELTA + HW)
  soundness:      d2_jk = b2_j - 2 a.t < B2C + HW + v_k - B2C - DELTA - HW
                        = v_k - DELTA  (true closer target)

Device layout (queries on PSUM partitions, targets on the free dim):
  2x4 grid: cores 0-3 take 1024 queries each over the first S/2 band targets;
  cores 4-7 the same query slices over the second S/2. Per core, 8 query
  blocks of 128; per block one PSUM tile [128, S_c]:
    psum[q, t] = sum_d T[t, d] * (-2 a[q, d])   (fp8 DoubleRow, 256-deep)
  then ONE fused instruction per tile yields the per-query witness measure:
    ACT: out = Relu(thr_q - psum), accum_out[q] = sum(out)   (>0 iff witness)
    DVE: out = (psum is_lt thr_q), accum_out[q] = count
  Only the [128, 8] accum table is DMA'd out (4KB/core).
"""
import os
import sys

for _p in ("/opt/trn_rl_repo", "/root/.axon_site/_ro/trn_rl_repo"):
    if os.path.isdir(_p) and _p not in sys.path:
        sys.path.append(_p)

from contextlib import ExitStack

import ml_dtypes
import numpy as np

NX, NY, D, K = 30000, 30000, 256, 4096
NCORES = 8
P = 128
DC = D // P          # 2 contraction k-tiles (fp8 DoubleRow: 256-deep)
NQ = 1024            # queries per core (cores c and c+4 share a query slice)
QB = NQ // P         # 8 query blocks
S_TOTAL = 1024       # sampled targets (device witness search set)
S_C = S_TOTAL // 2   # sampled targets per core (two halves)
DELTA = 18.5         # witness margin >= device arithmetic error bound
ACT_SET = (1, 3, 5, 7)  # query blocks routed through ScalarE

_CACHE: dict = {}


def _build_nc():
    import concourse.tile as tile
    from concourse import bacc, mybir
    nc = bacc.Bacc("TRN2", target_bir_lowering=False)
    # at[p, half, dc, q]: query halves outermost so each half's DMA is
    # contiguous per partition (1KB runs, full DMA speed)
    at_d = nc.dram_tensor(
        "at", [P, 2, DC, NQ // 2], mybir.dt.float8e4, kind="ExternalInput"
    )
    tt_d = nc.dram_tensor("tt", [P, DC, S_C], mybir.dt.float8e4, kind="ExternalInput")
    vb_d = nc.dram_tensor("vb", [P, QB], mybir.dt.float32, kind="ExternalInput")
    accw_d = nc.dram_tensor("accw", [P, QB], mybir.dt.float32, kind="ExternalOutput")

    with tile.TileContext(nc) as tc:
        with ExitStack() as ctx:
            sb = ctx.enter_context(tc.tile_pool(name="sb", bufs=1))
            dump = ctx.enter_context(tc.tile_pool(name="dump", bufs=3))
            nbanks = max(1, S_C // 512)
            psum = ctx.enter_context(
                tc.tile_pool(name="psum", bufs=8 // nbanks, space="PSUM")
            )

            # Inputs spread across the three DMA-capable queues (SP, gpsimd,
            # Activation), ordered so the first matmul's deps land first.
            tt = sb.tile([P, DC, S_C], mybir.dt.float8e4)
            nc.sync.dma_start(tt[:], tt_d[:])
            at = sb.tile([P, 2, DC, NQ // 2], mybir.dt.float8e4)
            nc.gpsimd.dma_start(at[:, 0], at_d[:, 0])
            vb = sb.tile([P, QB], mybir.dt.float32)
            nc.sync.dma_start(vb[:], vb_d[:])
            accw = sb.tile([P, QB], mybir.dt.float32)
            nc.gpsimd.memset(accw[:], 0.0)
            warm = sb.tile([P, 1], mybir.dt.float32)
            nc.gpsimd.memset(warm[:], 0.0)
            zz = sb.tile([P, S_C], mybir.dt.bfloat16)
            nc.vector.memset(zz[:], 0.0)

            # Pull the ACT table load off the critical path while DMAs fly;
            # the second half of `at` (query blocks 4-7) follows it on the
            # Activation queue.
            nc.scalar.activation(
                warm[:], warm[:], mybir.ActivationFunctionType.Relu,
                bias=0.0, scale=1.0,
            )
            nc.scalar.dma_start(at[:, 1], at_d[:, 1])

            for qb in range(QB):
                ps = psum.tile([P, S_C], mybir.dt.float32)
                for h in range(max(1, S_C // 512)):
                    t0 = h * 512
                    tw = min(512, S_C)
                    nc.tensor.matmul(
                        ps[:, t0:t0 + tw],
                        at[:, qb // 4, :, (qb % 4) * P:(qb % 4 + 1) * P],
                        tt[:, :, t0:t0 + tw],
                        start=True, stop=True,
                        perf_mode=mybir.MatmulPerfMode.DoubleRow,
                    )
                vo = dump.tile([P, S_C], mybir.dt.float16, tag="vo")
                if qb in ACT_SET:
                    # accum[q] = sum_t relu(thr_q - psum) : > 0 iff witness
                    nc.scalar.activation(
                        vo[:], ps[:], mybir.ActivationFunctionType.Relu,
                        bias=vb[:, qb:qb + 1], scale=-1.0,
                        accum_out=accw[:, qb:qb + 1],
                    )
                else:
                    # accum[q] = #targets with (psum - thr_q) < 0
                    nc.vector.scalar_tensor_tensor(
                        vo[:], ps[:], vb[:, qb:qb + 1], zz[:],
                        op0=mybir.AluOpType.subtract,
                        op1=mybir.AluOpType.is_lt,
                        accum_out=accw[:, qb:qb + 1],
                    )
            nc.sync.dma_start(accw_d[:], accw[:])

    nc.compile()
    return nc


def _get_nc():
    if "nc" not in _CACHE:
        _CACHE["nc"] = _build_nc()
    return _CACHE["nc"]


def _marshal(mapped, target, idx):
    """Host-side sharding/quantization. Returns (in_maps, a, b2_64)."""
    a = mapped[idx]                                   # [K, D] fp32
    at_all = np.ascontiguousarray((-2.0 * a).T)       # [D, K]

    b2_64 = (target.astype(np.float64) ** 2).sum(1)   # exact fp64 row norms
    med = np.median(b2_64)
    sidx = np.sort(np.argsort(np.abs(b2_64 - med))[:S_TOTAL])
    _CACHE["sidx"] = sidx
    b2band = b2_64[sidx]
    b2c = float(b2band.mean())
    hw = float(np.abs(b2band - b2c).max())            # band halfwidth
    _CACHE["band"] = (b2c, hw)
    tsub = target[sidx]                               # [S, D]

    # tt[p, dc, t] = tsub[t, dc*128 + p] in fp8
    tt_all = np.ascontiguousarray(
        tsub.reshape(S_TOTAL, DC, P).transpose(2, 1, 0)
    ).astype(ml_dtypes.float8_e4m3)                   # [P, DC, S]
    tt_half = [np.ascontiguousarray(tt_all[:, :, :S_C]),
               np.ascontiguousarray(tt_all[:, :, S_C:])]

    # v_k = d2 at own index (exact); thr = v - b2c - (DELTA + hw)
    v = b2_64[idx] - 2.0 * np.einsum(
        "kd,kd->k", a.astype(np.float64), target[idx].astype(np.float64)
    )
    _CACHE["v"] = v
    thr_all = (v - b2c - (DELTA + hw)).astype(np.float32)

    at_cores, vb_cores = [], []
    for cq in range(K // NQ):                          # 4 query slices
        sl = slice(cq * NQ, (cq + 1) * NQ)
        # at[p, half, dc, q'] = at_all[dc*128+p, cq*NQ + half*512 + q']
        at_cores.append(np.ascontiguousarray(
            at_all[:, sl].reshape(DC, P, 2, NQ // 2).transpose(1, 2, 0, 3)
        ).astype(ml_dtypes.float8_e4m3))               # [P, 2, DC, NQ//2]
        vb_cores.append(np.ascontiguousarray(thr_all[sl].reshape(QB, P).T))

    in_maps = []
    for c in range(NCORES):
        half, cq = c // 4, c % 4
        in_maps.append({
            "at": at_cores[cq], "tt": tt_half[half], "vb": vb_cores[cq],
        })
    return in_maps, a, b2_64


def kernel(mapped: np.ndarray, target: np.ndarray, indexes: np.ndarray) -> np.ndarray:
    from concourse.bass_utils import run_bass_kernel_spmd

    mapped = np.asarray(mapped, dtype=np.float32)
    target = np.asarray(target, dtype=np.float32)
    idx = np.asarray(indexes).astype(np.int64)

    in_maps, a, b2_64 = _marshal(mapped, target, idx)

    # ---- run on the 8 NeuronCores (host numpy fallback if the device path
    # fails repeatedly — correctness insurance) ----
    witness = None
    last_exc = None
    for attempt in range(3):
        try:
            nc = _get_nc()
            kwargs = {}
            if os.environ.get("KERNEL_TRACE_DIR"):
                kwargs["tmpdir"] = os.environ["KERNEL_TRACE_DIR"]
            res = run_bass_kernel_spmd(
                nc, in_maps, core_ids=list(range(NCORES)), **kwargs
            )
            _CACHE["last_res"] = res  # exec_time_ns/profile when BASS_TRACE=1
            # accw[p, qb] on core c: measure for query (c%4)*1024 + qb*128 + p
            # over the sampled-target half c//4
            w = np.zeros(K, dtype=np.float64)
            for c in range(NCORES):
                acc = res.results[c]["accw"].astype(np.float64)  # [P, QB]
                cq = c % 4
                w[cq * NQ:(cq + 1) * NQ] += acc.T.reshape(NQ)
            witness = w > 0.0
            break
        except Exception as e:  # noqa: BLE001 - retry/fallback on any device error
            last_exc = e
            _CACHE.pop("nc", None)
    if witness is None:
        sys.stderr.write(f"kernel: device path failed ({last_exc}); host fallback\n")
        witness = np.zeros(K, dtype=bool)

    # ---- host decision: witnessed queries are proven mismatched; the rest
    # get an exact fp64 check ----
    mismatch = witness.copy()
    flagged = np.nonzero(~witness)[0]
    _CACHE["flagged_n"] = len(flagged)
    t64 = None
    for i in range(0, len(flagged), 64):
        blk = flagged[i:i + 64]
        if t64 is None:
            t64 = target.astype(np.float64)
        d2 = b2_64[None, :] - 2.0 * (a[blk].astype(np.float64) @ t64.T)
        mismatch[blk] = np.argmin(d2, axis=1) != idx[blk]

    return np.asarray(mismatch.mean(), dtype=np.float32)


if __name__ == "__main__":
    rng = np.random.default_rng(1)
    mapped = rng.standard_normal((NX, D)).astype(np.float32)
    target = rng.standard_normal((NY, D)).astype(np.float32)
    indexes = rng.integers(0, NY, size=K).astype(np.int32)
    out = kernel(mapped=mapped, target=target, indexes=indexes)
    print("kernel output:", out, out.shape, out.dtype)
